# revision 42
# baseline (speedup 1.0000x reference)
"""L-infinity distance "convolution" kernel for Trainium2 (8 NeuronCores).

Computes out[b, co, h, w] = max_acc |weights[co, acc] - patch[b, h, w, acc]| + bias[co]
where patches are 3x3 replicate-padded windows over x (4, 16, 64, 64),
acc = (c, kh, kw) ordered, accl = 16*9 = 144, cout = 64.

Sharding: 8 cores = 4 batches x 2 row-halves. Each core computes a
[2048 positions, 64 cout] shard. No collectives needed.

Device layout: partitions = 128 spatial positions per tile (16 tiles/core),
weights replicated across partitions (one broadcast DMA per cout chunk).
Two compute paths, interleaved so both engines stay saturated:
  - DVE path: a custom SEGMENTED scan-max DVE op (hand-lowered 3-state uop
    FSM whose SUB_DIM_DONE step state re-seeds the scan recurrence at each
    [P, S, N] page boundary). One instruction covers a whole cout-chunk:
    in0 = weights [P, S=32 couts, 144], in1 = patch row page-broadcast,
    out through an AP whose inner dim has step 0 ([[1, S], [0, 144]]) so
    each page's final running max lands directly in dist[:, co]. This is
    1 cycle/element with ~1% instruction overhead, fp32-exact.
  - GPSIMD path (CFG["gps_count"] of 16 tiles): big tensor_tensor subtract
    on gpsimd, then a segmented tensor_reduce(max, abs) on DVE + bias add
    on gpsimd.

All arithmetic is fp32 and bit-exact vs the float32 reference (the custom
op computes max(a-b, b-a) = |a-b| exactly; reductions are exact maxes).
TimelineSim cost model: ~176 us per core (vs ~331 us for the naive
TT-subtract + segmented-reduce version, ~237 us for the one-instruction-
per-cout scan variant, ~202 us for that plus gpsimd offload).
"""

import numpy as np

B, C, H, W = 4, 16, 64, 64
K = 3
COUT = 64
ACC = C * K * K  # 144
HOUT, WOUT = 64, 64
NPOS = HOUT * WOUT  # 4096
NCORES = 8
HALVES = 2
POS_PER_CORE = NPOS // HALVES  # 2048
P = 128  # partitions
NTILES = POS_PER_CORE // P  # 16
COG = 8  # cout chunk for weight broadcast tiles
NCHUNK = COUT // COG  # 8

# tuning knobs (A/B-tested via TimelineSim)
CFG = {
    "gps_count": 7,  # how many of the 16 tiles go to gpsimd
    "gps_tree": 0,  # (unused) legacy knob
    "mix_tiles": 0,  # squash tiles that donate their last cout-chunk to gpsimd
    "gps_whole_w": False,  # gps sub as one op reading a whole-weights tile
    "gps_full": False,  # gps tiles also run the absmax tree on gpsimd (loses)
    "gps_bias": True,  # run the gps tiles' bias add on gpsimd instead of DVE
    "dwork_bufs": 3,
    "work_bufs": 6,
    "outp_bufs": 6,
    "w_cog": 32,  # cout per weight-chunk tile (32 -> 2 chunks)
    "flush_keep": 0,  # pending gpsimd reduces to hold back at each flush
}

_TRACE = False

_OP_CACHE = None


def _lower_segscan(spec, ver):
    """Hand-lowered 3-state FSM for a SEGMENTED scan: seed -> steady, with a
    SUB_DIM_DONE step state that re-seeds the scan recurrence on the first
    element of each [P, S, N] page (computing op(init, expr) instead of
    op(carry, expr)). The stock lower() has no per-page reset for regular
    scans; this provides one, giving per-page reductions from one
    instruction. HW-verified bit-exact."""
    import concourse.dve_spec as ds
    from concourse.dve_spec import Trigger

    n_lanes, n_stages = ds.N_LANES[ver], ds.N_STAGES[ver]
    ds._validate_body(spec, ver)
    spec2 = ds._hoist_stream_invariant_ops(spec)
    scans = ds._collect(spec2.body, ds.Scan)
    latches = ds._collect(spec2.body, ds.Latch)
    assert not latches and spec2.accum is None
    p = ds._build_placement(spec2, scans, n_stages, n_lanes)
    seed_ov, step_ov0 = ds._scan_overrides(scans, p.node_stage)
    assert not step_ov0  # regular scans only (no PageIdx)
    step_ov = {}
    for sc in scans:
        d = p.node_stage[sc]
        step_ov[d] = ds._Stage(sc.op, ds._scan_init(sc), sc.expr)
    body_lvs = ds._body_scan_leaves(spec2)
    consume = (ds.Src0 in body_lvs, ds.Src1 in body_lvs)
    states = [
        ds._State(
            placement=p,
            overrides=seed_ov,
            trigger=ds.COUNT_ONCE,
            repeat=1,
            next=(1, 0, 0),
            write_out=False,
        ),
        ds._State(
            placement=p,
            consume=consume,
            trigger=(Trigger.SRC_TENSOR_DONE, Trigger.SUB_DIM_DONE, Trigger.NONE),
            next=(0, 2, 0),
        ),
        ds._State(
            placement=p,
            consume=consume,
            overrides=step_ov,
            trigger=(Trigger.SRC_TENSOR_DONE, Trigger.SUB_DIM_DONE, Trigger.COUNT),
            next=(0, 2, 1),
            repeat=1,
        ),
    ]
    out = [ds._assemble(s) for s in states]
    for u in out:
        u.validate(ver)
    return out


def _get_op():
    """Register (once) the segmented |a-b| scan-max custom DVE op."""
    global _OP_CACHE
    if _OP_CACHE is not None:
        return _OP_CACHE
    from concourse.dve_spec import Spec, Src0, Src1, maxx, AluOp, scan
    from concourse.dve_uop import DveOpSpec
    import concourse.dve_ops as dve_ops
    from concourse.dve_ops import DveOp

    def _ref(in0, in1, s0, s1, imm2):
        b = np.maximum.accumulate(np.abs(in0.astype(np.float32) - in1), axis=-1)
        return b.astype(np.float32)

    spec = Spec(body=scan(AluOp.MAX, maxx(Src0 - Src1, Src1 - Src0)), reference=_ref)
    name = "ABSDIFF_MAX_SEGSCAN"
    if name not in dve_ops._SUB_OPCODE_FOR_NAME:
        row = max(dve_ops._SUB_OPCODE_FOR_NAME.values()) + 1
        assert row < 0x20
        dve_ops._SUB_OPCODE_FOR_NAME[name] = row
    row = dve_ops._SUB_OPCODE_FOR_NAME[name]
    shas = {}
    for ver in ("v3", "v4"):
        s = DveOpSpec(
            name=name, opcode=row, uops=_lower_segscan(spec, ver), rd1_en=True
        )
        # Pre-populate the compile cache so DveOp.compile() returns the
        # hand-lowered program instead of re-running the stock lower().
        dve_ops._COMPILE_CACHE[(name, ver)] = s
        shas[ver] = s.sha(ver)
    op = DveOp(name, spec, subdim=True, uops_sha=shas)
    if all(o.name != name for o in dve_ops.OPS):
        dve_ops.OPS.append(op)
        dve_ops.CUSTOM_DVE_SPECS[name] = spec
    _OP_CACHE = op
    return op


def _build_bass():
    import concourse.bass as bass
    import concourse.bacc as bacc
    import concourse.mybir as mybir
    import concourse.tile as tile
    from concourse.alu_op_type import AluOpType

    op = _get_op()

    nc = bacc.Bacc("TRN2", target_bir_lowering=False, debug=False, num_devices=NCORES)
    patches_d = nc.dram_tensor(
        "patches", [POS_PER_CORE, ACC], mybir.dt.float32, kind="ExternalInput"
    )
    w_d = nc.dram_tensor("w", [1, COUT * ACC], mybir.dt.float32, kind="ExternalInput")
    bias_d = nc.dram_tensor("bias", [1, COUT], mybir.dt.float32, kind="ExternalInput")
    out_d = nc.dram_tensor(
        "out", [POS_PER_CORE, COUT], mybir.dt.float32, kind="ExternalOutput"
    )

    # gpsimd tiles interleaved with DVE tiles; odd positions first so the
    # DVE starts on tile 0 immediately.
    kg = CFG["gps_count"]
    order = list(range(1, NTILES, 2)) + list(range(0, NTILES, 2))
    gps_tiles = set(order[:kg])
    tree_levels = CFG["gps_tree"]
    # squash tiles whose last cout-group is donated to gpsimd
    squash_order = [t for t in order if t not in gps_tiles]
    mix_tiles = set(squash_order[: CFG["mix_tiles"]])

    with tile.TileContext(nc) as tc:
        with (
            tc.tile_pool(name="consts", bufs=1) as consts,
            tc.tile_pool(name="work", bufs=CFG["work_bufs"]) as work,
            tc.tile_pool(name="dwork", bufs=CFG["dwork_bufs"]) as dwork,
            tc.tile_pool(name="outp", bufs=CFG["outp_bufs"]) as outp,
        ):
            # weights replicated across partitions, in separate chunk tiles
            # so early compute only waits on its own chunk's DMA
            cog = CFG["w_cog"]
            nch = COUT // cog
            wchunks = []
            for g in range(nch):
                wt = consts.tile([P, cog * ACC], mybir.dt.float32, tag=f"wch{g}")
                base = w_d[0:1, g * cog * ACC : (g + 1) * cog * ACC]
                src = bass.AP(
                    tensor=base.tensor, offset=base.offset, ap=[[0, P], [1, cog * ACC]]
                )
                nc.sync.dma_start(out=wt[:, :], in_=src)
                wchunks.append(wt)
            wbig = None
            if gps_tiles and CFG["gps_whole_w"]:
                wbig = consts.tile([P, COUT * ACC], mybir.dt.float32)
                base = w_d[0:1, :]
                src = bass.AP(
                    tensor=base.tensor, offset=base.offset, ap=[[0, P], [1, COUT * ACC]]
                )
                nc.sync.dma_start(out=wbig[:, :], in_=src)
            bias_rep = consts.tile([P, COUT], mybir.dt.float32)
            bbase = bias_d[0:1, :]
            bsrc = bass.AP(
                tensor=bbase.tensor, offset=bbase.offset, ap=[[0, P], [1, COUT]]
            )
            nc.sync.dma_start(out=bias_rep[:, :], in_=bsrc)

            pending = []  # gpsimd tiles awaiting their DVE reduce

            def flush_pending(keep=0):
                while len(pending) > keep:
                    t0, r3 = pending.pop(0)
                    dist = outp.tile([P, COUT], mybir.dt.float32, tag="dist")
                    nc.vector.tensor_reduce(
                        out=dist[:, :],
                        in_=r3,
                        axis=mybir.AxisListType.X,
                        op=AluOpType.max,
                        apply_absolute_value=True,
                    )
                    bias_eng = nc.gpsimd if CFG["gps_bias"] else nc.vector
                    bias_eng.tensor_tensor(
                        out=dist[:, :],
                        in0=dist[:, :],
                        in1=bias_rep[:, :],
                        op=AluOpType.add,
                    )
                    nc.sync.dma_start(
                        out=out_d[t0 * P : (t0 + 1) * P, :], in_=dist[:, :]
                    )

            for t in range(NTILES):
                pt = work.tile([P, ACC], mybir.dt.float32, tag="pt")
                nc.sync.dma_start(out=pt[:, :], in_=patches_d[t * P : (t + 1) * P, :])
                if t in gps_tiles:
                    # chunked subtract: one gpsimd op per cout-group, reading
                    # its weight chunk + the patch broadcast
                    d_t = dwork.tile([P, COUT * ACC], mybir.dt.float32, tag="D")
                    if wbig is not None:
                        pt_b = pt[:, :].unsqueeze(1).broadcast_to([P, COUT, ACC])
                        nc.gpsimd.tensor_tensor(
                            out=d_t[:, :].rearrange("p (c a) -> p c a", a=ACC),
                            in0=wbig[:, :].rearrange("p (c a) -> p c a", a=ACC),
                            in1=pt_b,
                            op=AluOpType.subtract,
                        )
                    else:
                        pt_b = pt[:, :].unsqueeze(1).broadcast_to([P, cog, ACC])
                        for g in range(nch):
                            d3g = d_t[
                                :, g * cog * ACC : (g + 1) * cog * ACC
                            ].rearrange("p (c a) -> p c a", a=ACC)
                            w3g = wchunks[g][:, :].rearrange("p (c a) -> p c a", a=ACC)
                            nc.gpsimd.tensor_tensor(
                                out=d3g, in0=w3g, in1=pt_b, op=AluOpType.subtract
                            )
                    if CFG["gps_full"]:
                        # absmax tree on gpsimd, ping-ponging between d_t and
                        # t2: 144 -> 72 -> 36 -> 18 -> 9, then only a 9-wide
                        # segmented reduce is left for the DVE
                        t2 = dwork.tile(
                            [P, COUT * (ACC // 2)], mybir.dt.float32, tag="T2"
                        )
                        src, aux = d_t, t2
                        w_cur = ACC
                        while w_cur > CFG.get("gps_stop_w", 9):
                            h = w_cur // 2
                            s3 = src[:, : COUT * w_cur].rearrange(
                                "p (c a) -> p c a", a=w_cur
                            )
                            o3 = aux[:, : COUT * h].rearrange(
                                "p (c a) -> p c a", a=h
                            )
                            nc.gpsimd.tensor_tensor(
                                out=o3,
                                in0=s3[:, :, 0:h],
                                in1=s3[:, :, h:w_cur],
                                op=AluOpType.abs_max,
                            )
                            src, aux = aux, src
                            w_cur = h
                        r3 = src[:, : COUT * w_cur].rearrange(
                            "p (c a) -> p c a", a=w_cur
                        )
                        pending.append((t, r3))
                    else:
                        pending.append(
                            (t, d_t[:, :].rearrange("p (c a) -> p c a", a=ACC))
                        )
                else:
                    # mix tiles donate their last weight chunk to gpsimd
                    n_sq = COUT - cog if t in mix_tiles else COUT
                    dist = outp.tile([P, COUT], mybir.dt.float32, tag="dist")
                    dm = None
                    pt_b = pt[:, :].unsqueeze(1).broadcast_to([P, cog, ACC])
                    if t in mix_tiles:
                        dm = dwork.tile([P, cog * ACC], mybir.dt.float32, tag="Dm")
                        w3g = wchunks[nch - 1][:, :].rearrange(
                            "p (c a) -> p c a", a=ACC
                        )
                        nc.gpsimd.tensor_tensor(
                            out=dm[:, :].rearrange("p (c a) -> p c a", a=ACC),
                            in0=w3g,
                            in1=pt_b,
                            op=AluOpType.subtract,
                        )
                    # segmented scan-max: one instruction per cout-chunk,
                    # whose step-0-inner output AP drops each page's final
                    # running max into dist[:, co]
                    for g in range(n_sq // cog):
                        d0 = dist[:, g * cog : (g + 1) * cog]
                        squash = bass.AP(
                            tensor=d0.tensor,
                            offset=d0.offset,
                            ap=[d0.ap[0], [1, cog], [0, ACC]],
                        )
                        w3 = wchunks[g][:, :].rearrange("p (c a) -> p c a", a=ACC)
                        nc.vector._custom_dve(op, out=squash, in0=w3, in1=pt_b)
                    nc.vector.tensor_tensor(
                        out=dist[:, 0:n_sq],
                        in0=dist[:, 0:n_sq],
                        in1=bias_rep[:, 0:n_sq],
                        op=AluOpType.add,
                    )
                    if dm is not None:
                        r3 = dm[:, :].rearrange("p (c a) -> p c a", a=ACC)
                        nc.vector.tensor_reduce(
                            out=dist[:, n_sq:COUT],
                            in_=r3,
                            axis=mybir.AxisListType.X,
                            op=AluOpType.max,
                            apply_absolute_value=True,
                        )
                        nc.vector.tensor_tensor(
                            out=dist[:, n_sq:COUT],
                            in0=dist[:, n_sq:COUT],
                            in1=bias_rep[:, n_sq:COUT],
                            op=AluOpType.add,
                        )
                    nc.sync.dma_start(
                        out=out_d[t * P : (t + 1) * P, :], in_=dist[:, :]
                    )
                    flush_pending(keep=CFG.get("flush_keep", 0))
            flush_pending()
    nc.compile()
    return nc


def _host_prep(inputs):
    x = np.asarray(inputs["x"], dtype=np.float32)
    weights = np.asarray(inputs["weights"], dtype=np.float32)
    bias = np.asarray(inputs["bias"], dtype=np.float32)
    assert x.shape == (B, C, H, W)
    assert weights.shape == (COUT, ACC)

    x_pad = np.pad(x, ((0, 0), (0, 0), (1, 1), (1, 1)), mode="edge")
    from numpy.lib.stride_tricks import sliding_window_view

    pw = sliding_window_view(x_pad, (K, K), axis=(2, 3))  # (B, C, HOUT, WOUT, K, K)
    patches = np.ascontiguousarray(pw.transpose(0, 2, 3, 1, 4, 5)).reshape(
        B, NPOS, ACC
    )
    wflat = np.ascontiguousarray(weights.reshape(1, COUT * ACC))
    bflat = np.ascontiguousarray(bias.reshape(1, COUT))
    return patches, wflat, bflat


_NC_CACHE = None


def _get_nc():
    global _NC_CACHE
    if _NC_CACHE is None:
        _NC_CACHE = _build_bass()
    return _NC_CACHE


def _run(inputs, trace=False):
    from concourse.bass_utils import run_bass_kernel_spmd

    patches, wflat, bflat = _host_prep(inputs)
    in_maps = []
    for core in range(NCORES):
        b, half = core // HALVES, core % HALVES
        shard = np.ascontiguousarray(
            patches[b, half * POS_PER_CORE : (half + 1) * POS_PER_CORE, :]
        )
        in_maps.append({"patches": shard, "w": wflat, "bias": bflat})

    nc = _get_nc()
    res = run_bass_kernel_spmd(nc, in_maps, core_ids=list(range(NCORES)), trace=trace)

    rows_per_half = POS_PER_CORE // WOUT  # 32
    out = np.empty((B, COUT, HOUT, WOUT), dtype=np.float32)
    for core in range(NCORES):
        b, half = core // HALVES, core % HALVES
        o = res.results[core]["out"]  # [POS_PER_CORE, COUT]
        out[b, :, half * rows_per_half : (half + 1) * rows_per_half, :] = o.T.reshape(
            COUT, rows_per_half, WOUT
        )
    return out, res


def kernel(**inputs) -> np.ndarray:
    out, _ = _run(inputs, trace=_TRACE)
    return out


# revision 52
# speedup vs baseline: 1.0596x; 1.0596x over previous
"""L-infinity distance "convolution" kernel for Trainium2 (8 NeuronCores).

Computes out[b, co, h, w] = max_acc |weights[co, acc] - patch[b, h, w, acc]| + bias[co]
where patches are 3x3 replicate-padded windows over x (4, 16, 64, 64),
acc = (c, kh, kw) ordered, accl = 16*9 = 144, cout = 64.

Sharding: 8 cores = 4 batches x 2 row-halves. Each core computes a
[2048 positions, 64 cout] shard. No collectives needed.

Device layout: partitions = 128 spatial positions per tile (16 tiles/core),
weights replicated across partitions (one broadcast DMA per cout chunk).
Two compute paths, interleaved so both engines stay saturated:
  - DVE path: a custom SEGMENTED scan-max DVE op (hand-lowered 3-state uop
    FSM whose SUB_DIM_DONE step state re-seeds the scan recurrence at each
    [P, S, N] page boundary). One instruction covers a whole cout-chunk:
    in0 = weights [P, S=32 couts, 144], in1 = patch row page-broadcast,
    out through an AP whose inner dim has step 0 ([[1, S], [0, 144]]) so
    each page's final running max lands directly in dist[:, co]. This is
    1 cycle/element with ~1% instruction overhead, fp32-exact.
  - GPSIMD path (CFG["gps_count"] of 16 tiles): big tensor_tensor subtract
    on gpsimd, then a segmented tensor_reduce(max, abs) on DVE + bias add
    on gpsimd.

All arithmetic is fp32 and bit-exact vs the float32 reference (the custom
op computes max(a-b, b-a) = |a-b| exactly; reductions are exact maxes).
TimelineSim cost model: ~176 us per core (vs ~331 us for the naive
TT-subtract + segmented-reduce version, ~237 us for the one-instruction-
per-cout scan variant, ~202 us for that plus gpsimd offload).
"""

import numpy as np

B, C, H, W = 4, 16, 64, 64
K = 3
COUT = 64
ACC = C * K * K  # 144
HOUT, WOUT = 64, 64
NPOS = HOUT * WOUT  # 4096
NCORES = 8
HALVES = 2
POS_PER_CORE = NPOS // HALVES  # 2048
P = 128  # partitions
NTILES = POS_PER_CORE // P  # 16
COG = 8  # cout chunk for weight broadcast tiles
NCHUNK = COUT // COG  # 8

# tuning knobs (A/B-tested via TimelineSim)
CFG = {
    "gps_count": 0,  # gpsimd sub tiles (0: all-DVE; gpsimd saves DVE nothing)
    "gps_tree": 0,  # (unused) legacy knob
    "mix_tiles": 0,  # squash tiles that donate their last cout-chunk to gpsimd
    "gps_whole_w": False,  # gps sub as one op reading a whole-weights tile
    "gps_full": False,  # gps tiles also run the absmax tree on gpsimd (loses)
    "gps_bias": True,  # run the gps tiles' bias add on gpsimd instead of DVE
    "sq_bias_gps": True,  # squash tiles' bias add on (idle) gpsimd
    "dwork_bufs": 3,
    "work_bufs": 6,
    "outp_bufs": 6,
    "w_cog": 32,  # cout per weight-chunk tile if w_chunks is None
    "w_chunks": [8, 24, 32],  # cout chunk ladder: small first for early start
    "flush_keep": 0,  # pending gpsimd reduces to hold back at each flush
}

_TRACE = False

_OP_CACHE = None


def _lower_segscan(spec, ver):
    """Hand-lowered 3-state FSM for a SEGMENTED scan: seed -> steady, with a
    SUB_DIM_DONE step state that re-seeds the scan recurrence on the first
    element of each [P, S, N] page (computing op(init, expr) instead of
    op(carry, expr)). The stock lower() has no per-page reset for regular
    scans; this provides one, giving per-page reductions from one
    instruction. HW-verified bit-exact."""
    import concourse.dve_spec as ds
    from concourse.dve_spec import Trigger

    n_lanes, n_stages = ds.N_LANES[ver], ds.N_STAGES[ver]
    ds._validate_body(spec, ver)
    spec2 = ds._hoist_stream_invariant_ops(spec)
    scans = ds._collect(spec2.body, ds.Scan)
    latches = ds._collect(spec2.body, ds.Latch)
    assert not latches and spec2.accum is None
    p = ds._build_placement(spec2, scans, n_stages, n_lanes)
    seed_ov, step_ov0 = ds._scan_overrides(scans, p.node_stage)
    assert not step_ov0  # regular scans only (no PageIdx)
    step_ov = {}
    for sc in scans:
        d = p.node_stage[sc]
        step_ov[d] = ds._Stage(sc.op, ds._scan_init(sc), sc.expr)
    body_lvs = ds._body_scan_leaves(spec2)
    consume = (ds.Src0 in body_lvs, ds.Src1 in body_lvs)
    states = [
        ds._State(
            placement=p,
            overrides=seed_ov,
            trigger=ds.COUNT_ONCE,
            repeat=1,
            next=(1, 0, 0),
            write_out=False,
        ),
        ds._State(
            placement=p,
            consume=consume,
            trigger=(Trigger.SRC_TENSOR_DONE, Trigger.SUB_DIM_DONE, Trigger.NONE),
            next=(0, 2, 0),
        ),
        ds._State(
            placement=p,
            consume=consume,
            overrides=step_ov,
            trigger=(Trigger.SRC_TENSOR_DONE, Trigger.SUB_DIM_DONE, Trigger.COUNT),
            next=(0, 2, 1),
            repeat=1,
        ),
    ]
    out = [ds._assemble(s) for s in states]
    for u in out:
        u.validate(ver)
    return out


def _get_op():
    """Register (once) the segmented |a-b| scan-max custom DVE op."""
    global _OP_CACHE
    if _OP_CACHE is not None:
        return _OP_CACHE
    from concourse.dve_spec import Spec, Src0, Src1, maxx, AluOp, scan
    from concourse.dve_uop import DveOpSpec
    import concourse.dve_ops as dve_ops
    from concourse.dve_ops import DveOp

    def _ref(in0, in1, s0, s1, imm2):
        b = np.maximum.accumulate(np.abs(in0.astype(np.float32) - in1), axis=-1)
        return b.astype(np.float32)

    spec = Spec(body=scan(AluOp.MAX, maxx(Src0 - Src1, Src1 - Src0)), reference=_ref)
    name = "ABSDIFF_MAX_SEGSCAN"
    if name not in dve_ops._SUB_OPCODE_FOR_NAME:
        row = max(dve_ops._SUB_OPCODE_FOR_NAME.values()) + 1
        assert row < 0x20
        dve_ops._SUB_OPCODE_FOR_NAME[name] = row
    row = dve_ops._SUB_OPCODE_FOR_NAME[name]
    shas = {}
    for ver in ("v3", "v4"):
        s = DveOpSpec(
            name=name, opcode=row, uops=_lower_segscan(spec, ver), rd1_en=True
        )
        # Pre-populate the compile cache so DveOp.compile() returns the
        # hand-lowered program instead of re-running the stock lower().
        dve_ops._COMPILE_CACHE[(name, ver)] = s
        shas[ver] = s.sha(ver)
    op = DveOp(name, spec, subdim=True, uops_sha=shas)
    if all(o.name != name for o in dve_ops.OPS):
        dve_ops.OPS.append(op)
        dve_ops.CUSTOM_DVE_SPECS[name] = spec
    _OP_CACHE = op
    return op


def _build_bass():
    import concourse.bass as bass
    import concourse.bacc as bacc
    import concourse.mybir as mybir
    import concourse.tile as tile
    from concourse.alu_op_type import AluOpType

    op = _get_op()

    nc = bacc.Bacc("TRN2", target_bir_lowering=False, debug=False, num_devices=NCORES)
    patches_d = nc.dram_tensor(
        "patches", [POS_PER_CORE, ACC], mybir.dt.float32, kind="ExternalInput"
    )
    w_d = nc.dram_tensor("w", [1, COUT * ACC], mybir.dt.float32, kind="ExternalInput")
    bias_d = nc.dram_tensor("bias", [1, COUT], mybir.dt.float32, kind="ExternalInput")
    out_d = nc.dram_tensor(
        "out", [POS_PER_CORE, COUT], mybir.dt.float32, kind="ExternalOutput"
    )

    # gpsimd tiles interleaved with DVE tiles; odd positions first so the
    # DVE starts on tile 0 immediately.
    kg = CFG["gps_count"]
    order = list(range(1, NTILES, 2)) + list(range(0, NTILES, 2))
    gps_tiles = set(order[:kg])
    tree_levels = CFG["gps_tree"]
    # squash tiles whose last cout-group is donated to gpsimd
    squash_order = [t for t in order if t not in gps_tiles]
    mix_tiles = set(squash_order[: CFG["mix_tiles"]])

    with tile.TileContext(nc) as tc:
        with (
            tc.tile_pool(name="consts", bufs=1) as consts,
            tc.tile_pool(name="work", bufs=CFG["work_bufs"]) as work,
            tc.tile_pool(name="dwork", bufs=CFG["dwork_bufs"]) as dwork,
            tc.tile_pool(name="outp", bufs=CFG["outp_bufs"]) as outp,
            tc.tile_pool(name="psum", bufs=4, space="PSUM") as psp,
        ):
            # Weights replicated across all 128 partitions. A partition-
            # broadcast DMA has ~6 us fixed latency and Tile merges the
            # chunk waits, stalling the first consumer ~19 us. Instead:
            # one small [1, N] DMA, then a K=1 ones-matmul broadcast on the
            # (otherwise idle) PE with PSUM->SBUF copies on the (otherwise
            # idle) ScalarE. fp32 x 1.0 through the PE is bitwise exact
            # (HW-verified). First chunk is consumable in ~8 us and the
            # rest pipelines behind compute.
            chunk_sizes = CFG["w_chunks"] or [CFG["w_cog"]] * (COUT // CFG["w_cog"])
            assert sum(chunk_sizes) == COUT
            starts = [sum(chunk_sizes[:i]) for i in range(len(chunk_sizes))]
            cog = chunk_sizes[-1]  # for the mix-tile donation path
            nch = len(chunk_sizes)
            # SWDGE queue for these so the patch-tile loads (HWDGE) don't
            # queue behind them
            wflat_s = consts.tile([1, COUT * ACC], mybir.dt.float32)
            nc.gpsimd.dma_start(out=wflat_s[:, :], in_=w_d[0:1, :])
            bflat_s = consts.tile([1, COUT], mybir.dt.float32)
            nc.gpsimd.dma_start(out=bflat_s[:, :], in_=bias_d[0:1, :])
            ones = consts.tile([1, P], mybir.dt.float32)
            nc.gpsimd.memset(ones[:, :], 1.0)
            bias_rep = consts.tile([P, COUT], mybir.dt.float32)

            MMN = 512
            wchunks = []
            for g in range(nch):
                sz = chunk_sizes[g]
                wt = consts.tile([P, sz * ACC], mybir.dt.float32, tag=f"wch{g}")
                cols = sz * ACC
                for j in range(0, cols, MMN):
                    nn = min(MMN, cols - j)
                    ps = psp.tile([P, MMN], mybir.dt.float32, tag="psb")
                    c0 = starts[g] * ACC + j
                    nc.tensor.matmul(
                        ps[:, 0:nn],
                        ones[:, :],
                        wflat_s[:, c0 : c0 + nn],
                        start=True,
                        stop=True,
                    )
                    nc.scalar.copy(out=wt[:, j : j + nn], in_=ps[:, 0:nn])
                wchunks.append(wt)
                if g == 0:
                    psb = psp.tile([P, MMN], mybir.dt.float32, tag="psb")
                    nc.tensor.matmul(
                        psb[:, 0:COUT], ones[:, :], bflat_s[:, :], start=True, stop=True
                    )
                    nc.scalar.copy(out=bias_rep[:, :], in_=psb[:, 0:COUT])
            wbig = None

            pending = []  # gpsimd tiles awaiting their DVE reduce

            def flush_pending(keep=0):
                while len(pending) > keep:
                    t0, r3 = pending.pop(0)
                    dist = outp.tile([P, COUT], mybir.dt.float32, tag="dist")
                    nc.vector.tensor_reduce(
                        out=dist[:, :],
                        in_=r3,
                        axis=mybir.AxisListType.X,
                        op=AluOpType.max,
                        apply_absolute_value=True,
                    )
                    bias_eng = nc.gpsimd if CFG["gps_bias"] else nc.vector
                    bias_eng.tensor_tensor(
                        out=dist[:, :],
                        in0=dist[:, :],
                        in1=bias_rep[:, :],
                        op=AluOpType.add,
                    )
                    nc.sync.dma_start(
                        out=out_d[t0 * P : (t0 + 1) * P, :], in_=dist[:, :]
                    )

            for t in range(NTILES):
                pt = work.tile([P, ACC], mybir.dt.float32, tag="pt")
                nc.sync.dma_start(out=pt[:, :], in_=patches_d[t * P : (t + 1) * P, :])
                if t in gps_tiles:
                    # chunked subtract: one gpsimd op per cout-group, reading
                    # its weight chunk + the patch broadcast
                    d_t = dwork.tile([P, COUT * ACC], mybir.dt.float32, tag="D")
                    if wbig is not None:
                        pt_b = pt[:, :].unsqueeze(1).broadcast_to([P, COUT, ACC])
                        nc.gpsimd.tensor_tensor(
                            out=d_t[:, :].rearrange("p (c a) -> p c a", a=ACC),
                            in0=wbig[:, :].rearrange("p (c a) -> p c a", a=ACC),
                            in1=pt_b,
                            op=AluOpType.subtract,
                        )
                    else:
                        for g in range(nch):
                            sz = chunk_sizes[g]
                            s0 = starts[g]
                            pt_b = pt[:, :].unsqueeze(1).broadcast_to([P, sz, ACC])
                            d3g = d_t[
                                :, s0 * ACC : (s0 + sz) * ACC
                            ].rearrange("p (c a) -> p c a", a=ACC)
                            w3g = wchunks[g][:, :].rearrange("p (c a) -> p c a", a=ACC)
                            nc.gpsimd.tensor_tensor(
                                out=d3g, in0=w3g, in1=pt_b, op=AluOpType.subtract
                            )
                    if CFG["gps_full"]:
                        # absmax tree on gpsimd, ping-ponging between d_t and
                        # t2: 144 -> 72 -> 36 -> 18 -> 9, then only a 9-wide
                        # segmented reduce is left for the DVE
                        t2 = dwork.tile(
                            [P, COUT * (ACC // 2)], mybir.dt.float32, tag="T2"
                        )
                        src, aux = d_t, t2
                        w_cur = ACC
                        while w_cur > CFG.get("gps_stop_w", 9):
                            h = w_cur // 2
                            s3 = src[:, : COUT * w_cur].rearrange(
                                "p (c a) -> p c a", a=w_cur
                            )
                            o3 = aux[:, : COUT * h].rearrange(
                                "p (c a) -> p c a", a=h
                            )
                            nc.gpsimd.tensor_tensor(
                                out=o3,
                                in0=s3[:, :, 0:h],
                                in1=s3[:, :, h:w_cur],
                                op=AluOpType.abs_max,
                            )
                            src, aux = aux, src
                            w_cur = h
                        r3 = src[:, : COUT * w_cur].rearrange(
                            "p (c a) -> p c a", a=w_cur
                        )
                        pending.append((t, r3))
                    else:
                        pending.append(
                            (t, d_t[:, :].rearrange("p (c a) -> p c a", a=ACC))
                        )
                else:
                    # mix tiles donate their last weight chunk to gpsimd
                    n_sq = COUT - cog if t in mix_tiles else COUT
                    dist = outp.tile([P, COUT], mybir.dt.float32, tag="dist")
                    dm = None
                    pt_b = pt[:, :].unsqueeze(1).broadcast_to([P, cog, ACC])
                    if t in mix_tiles:
                        dm = dwork.tile([P, cog * ACC], mybir.dt.float32, tag="Dm")
                        w3g = wchunks[nch - 1][:, :].rearrange(
                            "p (c a) -> p c a", a=ACC
                        )
                        nc.gpsimd.tensor_tensor(
                            out=dm[:, :].rearrange("p (c a) -> p c a", a=ACC),
                            in0=w3g,
                            in1=pt_b,
                            op=AluOpType.subtract,
                        )
                    # segmented scan-max: one instruction per cout-chunk,
                    # whose step-0-inner output AP drops each page's final
                    # running max into dist[:, co]
                    for g in range(nch):
                        sz = chunk_sizes[g]
                        s0 = starts[g]
                        if s0 + sz > n_sq:
                            break
                        d0 = dist[:, s0 : s0 + sz]
                        squash = bass.AP(
                            tensor=d0.tensor,
                            offset=d0.offset,
                            ap=[d0.ap[0], [1, sz], [0, ACC]],
                        )
                        w3 = wchunks[g][:, :].rearrange("p (c a) -> p c a", a=ACC)
                        ptb = pt[:, :].unsqueeze(1).broadcast_to([P, sz, ACC])
                        nc.vector._custom_dve(op, out=squash, in0=w3, in1=ptb)
                    sq_bias_eng = (
                        nc.gpsimd if CFG.get("sq_bias_gps", False) else nc.vector
                    )
                    sq_bias_eng.tensor_tensor(
                        out=dist[:, 0:n_sq],
                        in0=dist[:, 0:n_sq],
                        in1=bias_rep[:, 0:n_sq],
                        op=AluOpType.add,
                    )
                    if dm is not None:
                        r3 = dm[:, :].rearrange("p (c a) -> p c a", a=ACC)
                        nc.vector.tensor_reduce(
                            out=dist[:, n_sq:COUT],
                            in_=r3,
                            axis=mybir.AxisListType.X,
                            op=AluOpType.max,
                            apply_absolute_value=True,
                        )
                        nc.vector.tensor_tensor(
                            out=dist[:, n_sq:COUT],
                            in0=dist[:, n_sq:COUT],
                            in1=bias_rep[:, n_sq:COUT],
                            op=AluOpType.add,
                        )
                    nc.sync.dma_start(
                        out=out_d[t * P : (t + 1) * P, :], in_=dist[:, :]
                    )
                    flush_pending(keep=CFG.get("flush_keep", 0))
            flush_pending()
    nc.compile()
    return nc


def _host_prep(inputs):
    x = np.asarray(inputs["x"], dtype=np.float32)
    weights = np.asarray(inputs["weights"], dtype=np.float32)
    bias = np.asarray(inputs["bias"], dtype=np.float32)
    assert x.shape == (B, C, H, W)
    assert weights.shape == (COUT, ACC)

    x_pad = np.pad(x, ((0, 0), (0, 0), (1, 1), (1, 1)), mode="edge")
    from numpy.lib.stride_tricks import sliding_window_view

    pw = sliding_window_view(x_pad, (K, K), axis=(2, 3))  # (B, C, HOUT, WOUT, K, K)
    patches = np.ascontiguousarray(pw.transpose(0, 2, 3, 1, 4, 5)).reshape(
        B, NPOS, ACC
    )
    wflat = np.ascontiguousarray(weights.reshape(1, COUT * ACC))
    bflat = np.ascontiguousarray(bias.reshape(1, COUT))
    return patches, wflat, bflat


_NC_CACHE = None


def _get_nc():
    global _NC_CACHE
    if _NC_CACHE is None:
        _NC_CACHE = _build_bass()
    return _NC_CACHE


def _run(inputs, trace=False):
    from concourse.bass_utils import run_bass_kernel_spmd

    patches, wflat, bflat = _host_prep(inputs)
    in_maps = []
    for core in range(NCORES):
        b, half = core // HALVES, core % HALVES
        shard = np.ascontiguousarray(
            patches[b, half * POS_PER_CORE : (half + 1) * POS_PER_CORE, :]
        )
        in_maps.append({"patches": shard, "w": wflat, "bias": bflat})

    nc = _get_nc()
    res = run_bass_kernel_spmd(nc, in_maps, core_ids=list(range(NCORES)), trace=trace)

    rows_per_half = POS_PER_CORE // WOUT  # 32
    out = np.empty((B, COUT, HOUT, WOUT), dtype=np.float32)
    for core in range(NCORES):
        b, half = core // HALVES, core % HALVES
        o = res.results[core]["out"]  # [POS_PER_CORE, COUT]
        out[b, :, half * rows_per_half : (half + 1) * rows_per_half, :] = o.T.reshape(
            COUT, rows_per_half, WOUT
        )
    return out, res


def kernel(**inputs) -> np.ndarray:
    out, _ = _run(inputs, trace=_TRACE)
    return out
